# revision 1
# baseline (speedup 1.0000x reference)
"""AttentionConvolution2D distributed kernel for 8 trn2 NeuronCores.

Sharding (per spec hint): data-parallel over batch (2) x spatial-parallel over
horizontal_blocks (4 slices of 16 block-cols), 1-block halo included in each
shard's input slice (no collectives needed). Small weights + positional
embedding replicated. Compute: bf16 matmuls (PE runs bf16 at 4x fp32 rate),
fp32 softmax.

Shapes (hardcoded per problem spec):
  x (2,128,256,256) f32, w_in (768,128), b_in (768,), w_out (128,256),
  b_out (128,), pce (8,256). Output (2,128,256,256) f32.
"""
import numpy as np

BS = 4
NH = 8
DH = 32
DIN = 128
DOUT = 128
H = 256
W = 256
B = 2
HB = W // BS   # 64 global block cols
VB = H // BS   # 64 block rows
NCORES = 8
WSLICES = 4
HBL = HB // WSLICES  # 16 block cols per shard
WL = HBL * BS        # 64 pixel cols per shard

_compiled = None


def _build():
    import jax
    import jax.numpy as jnp

    scale = 1.0 / np.sqrt(np.float32(DH))

    def shard_fn(x_s, w_in, b_in, w_out, b_out, bias, cut):
        # x_s: (DIN, H, WL+8) fp32 (halo-padded width slice)
        # bias: (NH, BS*BS, 9*BS*BS) fp32 ; cut: (HBL, VB, 9*BS*BS) fp32
        xb = jnp.swapaxes(x_s, 0, 2)  # (WL+8, H, DIN)
        xb = xb.reshape(HBL + 2, BS, VB, BS, DIN).transpose(0, 2, 1, 3, 4)
        # (hb_l=18, vb, bs_w, bs_h, DIN)
        xb16 = xb.astype(jnp.bfloat16)
        w16 = w_in.astype(jnp.bfloat16)
        proj = jax.lax.dot_general(
            xb16, w16.T, (((4,), (0,)), ((), ())),
            preferred_element_type=jnp.float32) + b_in
        # (18, vb, 4, 4, 768)
        pq = proj[1:HBL + 1, ..., :NH * DH]          # (16, vb, 4, 4, 256)
        pkv = proj[..., NH * DH:]                    # (18, vb, 4, 4, 512)
        pkv = jnp.pad(pkv, ((0, 0), (1, 1), (0, 0), (0, 0), (0, 0)))
        # (18, vb+2, 4, 4, 512)
        w16o = w_out.astype(jnp.bfloat16).T          # (256, 128)

        def step(j):
            sl = jax.lax.dynamic_slice(
                pkv, (j, 0, 0, 0, 0), (3, VB + 2, BS, BS, 2 * NH * DH))
            kvw = jnp.concatenate((sl[0], sl[1], sl[2]), axis=1)
            # (vb+2, 12, 4, 512)
            kvh = jnp.concatenate(
                (kvw[:-2], kvw[1:-1], kvw[2:]), axis=2)   # (vb, 12, 12, 512)
            kvh = kvh.reshape(VB, 9 * BS * BS, 2 * NH * DH)
            k = kvh[..., :NH * DH].reshape(VB, 9 * BS * BS, NH, DH)
            v = kvh[..., NH * DH:].reshape(VB, 9 * BS * BS, NH, DH)
            q = jax.lax.dynamic_slice(
                pq, (j, 0, 0, 0, 0), (1, VB, BS, BS, NH * DH))[0]
            q = q.reshape(VB, BS * BS, NH, DH)
            s = jnp.einsum('vqhd,vkhd->vhqk', q.astype(jnp.bfloat16),
                           k.astype(jnp.bfloat16),
                           preferred_element_type=jnp.float32)
            cj = jax.lax.dynamic_slice(cut, (j, 0, 0), (1, VB, 9 * BS * BS))
            s = s * scale - bias[None] - cj[0][:, None, None, :]
            s = s - jax.lax.stop_gradient(jnp.max(s, axis=-1, keepdims=True))
            e = jnp.exp(s)
            a = e / jnp.sum(e, axis=-1, keepdims=True)
            o = jnp.einsum('vhqk,vkhd->vqhd', a.astype(jnp.bfloat16),
                           v.astype(jnp.bfloat16),
                           preferred_element_type=jnp.float32)
            o = o.reshape(VB, BS * BS, NH * DH)
            o = jax.lax.dot_general(
                o.astype(jnp.bfloat16), w16o, (((2,), (0,)), ((), ())),
                preferred_element_type=jnp.float32) + b_out
            return o  # (vb, 16, 128)

        out = jax.lax.map(step, jnp.arange(HBL))  # (16, vb, 16, 128)
        out = out.reshape(HBL, VB, BS, BS, DOUT)
        # token (j, i, pw, ph) -> out[c, i*4+ph, j*4+pw]
        out = out.transpose(4, 1, 3, 0, 2).reshape(DOUT, H, WL)
        return out

    return jax, jnp, jax.pmap(shard_fn)


def _masks(pce):
    # bias[h, p, k] = pce[h, 16*(kw+4-pw) + (kh+4-ph)]
    a0 = np.arange(BS)
    a1 = np.arange(3 * BS)
    net = a1[None, :] + BS - a0[:, None]              # (4, 12)
    idx = (4 * BS * net)[:, None, :, None] + net[None, :, None, :]
    idx = idx.reshape(BS * BS, 9 * BS * BS)           # (16, 144)
    bias = pce[:, idx].astype(np.float32)             # (NH, 16, 144)
    # cut per shard: (WSLICES, HBL, VB, 144)
    ex = np.zeros((HB, 3 * BS), bool)
    ex[0, :BS] = True
    ex[-1, 2 * BS:] = True
    ey = np.zeros((VB, 3 * BS), bool)
    ey[0, :BS] = True
    ey[-1, 2 * BS:] = True
    blocked = ex[:, None, :, None] | ey[None, :, None, :]  # (HB, VB, 12, 12)
    cut = np.where(blocked, np.float32(30000.0), np.float32(0.0))
    cut = cut.reshape(HB, VB, 9 * BS * BS)
    return bias, cut.reshape(WSLICES, HBL, VB, 9 * BS * BS)


def kernel(x, w_in, b_in, w_out, b_out, pce):
    global _compiled
    if _compiled is None:
        _compiled = _build()
    jax, jnp, pfn = _compiled

    x = np.asarray(x, np.float32)
    bias, cuts = _masks(np.asarray(pce, np.float32))

    # halo-padded width slices: shard s = (batch s//4, wslice s%4)
    xp = np.pad(x, ((0, 0), (0, 0), (0, 0), (BS, BS)))
    xs = np.stack([
        xp[s // WSLICES, :, :, (s % WSLICES) * WL:(s % WSLICES) * WL + WL + 2 * BS]
        for s in range(NCORES)])                      # (8, 128, 256, 72)
    rep = lambda a: np.broadcast_to(np.asarray(a, np.float32),
                                    (NCORES,) + np.shape(a))
    cs = np.stack([cuts[s % WSLICES] for s in range(NCORES)])

    outs = np.asarray(pfn(xs, rep(w_in), rep(b_in), rep(w_out), rep(b_out),
                          rep(bias), cs))            # (8, 128, 256, 64)
    out = np.empty((B, DOUT, H, W), np.float32)
    for s in range(NCORES):
        out[s // WSLICES, :, :, (s % WSLICES) * WL:(s % WSLICES + 1) * WL] = outs[s]
    return out
